# revision 14
# baseline (speedup 1.0000x reference)
"""DescriptorLoss kernel for Trainium2 (8 NeuronCores, SPMD data-parallel).

Math:
    d[b,ij,kl] = sum_c desc0[b,c,ij] * desc1[b,c,kl]
    loss = mean(where(mask, 250*relu(1 - d), relu(d - 0.2)))

Per core: shard = (batch, i-slab) -> 1024 ij rows x 4096 kl cols, as 8 row
groups of 128 x 4096, each split into two 2048-col half-groups (hg = one
4-bank PSUM tile). The PE computes d' = 5*d with fp8e4m3 matmuls (same PE
rate as bf16, half the DMA/SBUF; quantization error ~2e-4 << tolerance) and
injects the mask with one extra matmul per 1024 cols:
    u = d' - 2048*m        (diag(-1) @ m8, m8 in {0, 2048} fp8e5m2)
With |d'| < ~1100 << 2048 the two hinges live in disjoint ranges of u, so
each needs only scalar constants. Every accumulating op on TRN2 runs at
1 elem/lane/cycle (no perf-mode uops for reductions), so the two hinge
sums stream straight from PSUM CONCURRENTLY on the two capable engines:
    ACT:  accA = sum relu(u - 1)        -> A = accA        (masked die)
    DVE:  accB = sum min(u, -2043)      -> B = -accB - 2043*N  (unmasked
                                            clamp; masked give relu(5-d'))
Host: loss = sum(A + 250*B)/5 / count. Accumulators are fp32 - the only
approximation anywhere is the fp8e4m3 descriptor quantization.
"""

import numpy as np
import ml_dtypes

import concourse.bacc as bacc
import concourse.mybir as mybir
import concourse.tile as tile
from concourse.bass_utils import run_bass_kernel_spmd

B, D, H, W = 2, 128, 64, 64
N_CORES = 8
IJ = H * W                 # 4096
ROWS_PER_CORE = IJ // 4    # 1024
G = ROWS_PER_CORE // 128   # 8 row groups of 128
HG_COLS = 2048             # half-group column width (one 4-bank PSUM tile)
N_HG = G * 2               # 16 half-groups per core
C = 2048.0                 # mask inject magnitude
THR = C - 5.0              # 2043: B-hinge clamp constant

MMF = 512                  # matmul moving free dim (cols per matmul)

_cached = {}


def _build_program():
    nc = bacc.Bacc("TRN2")
    f32 = mybir.dt.float32
    bf16 = mybir.dt.bfloat16
    f8e4 = mybir.dt.float8e4
    f8e5 = mybir.dt.float8e5
    Alu = mybir.AluOpType
    Act = mybir.ActivationFunctionType

    a5 = nc.declare_dram_parameter("a5", [D, ROWS_PER_CORE], f8e4, isOutput=False)
    bm = nc.declare_dram_parameter("bm", [D, IJ], f8e4, isOutput=False)
    idn = nc.declare_dram_parameter("idn", [D, D], f8e5, isOutput=False)
    m8 = nc.declare_dram_parameter("m8", [128, N_HG, HG_COLS], f8e5, isOutput=False)
    accs_out = nc.declare_dram_parameter("accs", [128, 2 * N_HG], f32, isOutput=True)

    with tile.TileContext(nc) as tc:
        with (
            tc.tile_pool(name="desc", bufs=1) as desc_pool,
            tc.tile_pool(name="mask", bufs=4) as mask_pool,
            tc.tile_pool(name="gouta", bufs=2) as ga_pool,
            tc.tile_pool(name="goutb", bufs=2) as gb_pool,
            tc.tile_pool(name="accs", bufs=1) as acc_pool,
            tc.tile_pool(name="psd", bufs=2, space="PSUM") as psum_pool,
        ):
            a5_t = desc_pool.tile([D, ROWS_PER_CORE], f8e4, tag="a5")
            bm_t = desc_pool.tile([D, IJ], f8e4, tag="bm")
            id_t = desc_pool.tile([D, D], f8e5, tag="idn")
            bias_a = desc_pool.tile([128, 1], f32, tag="ba")
            nc.sync.dma_start(a5_t[:], a5[:])
            nc.sync.dma_start(bm_t[:], bm[:])
            nc.sync.dma_start(id_t[:], idn[:])
            nc.gpsimd.memset(bias_a[:], -1.0)

            accA_t = acc_pool.tile([128, N_HG], f32, tag="accsA")
            accB_t = acc_pool.tile([128, N_HG], f32, tag="accsB")

            for hg in range(N_HG):
                g, h = hg // 2, hg % 2
                c0 = h * HG_COLS
                rs = slice(g * 128, (g + 1) * 128)

                mm_t = mask_pool.tile([128, HG_COLS], f8e5, tag="m8")
                nc.sync.dma_start(mm_t[:], m8[:, hg, :])

                psum_t = psum_pool.tile([128, HG_COLS], f32, tag="d")
                for j in range(HG_COLS // MMF):
                    js = slice(j * MMF, (j + 1) * MMF)
                    cs = slice(c0 + j * MMF, c0 + (j + 1) * MMF)
                    nc.tensor.matmul(
                        psum_t[:, js], a5_t[:, rs], bm_t[:, cs],
                        start=True, stop=False,
                    )
                for j in range(HG_COLS // MMF):
                    js = slice(j * MMF, (j + 1) * MMF)
                    nc.tensor.matmul(
                        psum_t[:, js], id_t[:], mm_t[:, js],
                        start=False, stop=True,
                    )

                g1 = ga_pool.tile([128, HG_COLS], bf16, tag="ga")
                g2 = gb_pool.tile([128, HG_COLS], bf16, tag="gb")
                nc.vector.tensor_scalar(
                    g2[:], psum_t[:], -THR, None,
                    op0=Alu.min, op1=Alu.add,
                    accum_out=accB_t[:, hg:hg + 1],
                )
                nc.scalar.activation(
                    g1[:], psum_t[:], Act.Relu,
                    bias=bias_a[:], scale=1.0,
                    accum_out=accA_t[:, hg:hg + 1],
                )

            nc.sync.dma_start(accs_out[:, :N_HG], accA_t[:])
            nc.sync.dma_start(accs_out[:, N_HG:], accB_t[:])

    nc.finalize()
    return nc


def _prep_inputs(descriptors_0, descriptors_1, similarity_mask):
    d0 = np.asarray(descriptors_0, dtype=np.float32)
    d1 = np.asarray(descriptors_1, dtype=np.float32)
    mkv = np.asarray(similarity_mask)
    idn = np.zeros((D, D), dtype=np.float32)
    np.fill_diagonal(idn, -1.0)
    idn = np.ascontiguousarray(idn.astype(ml_dtypes.float8_e5m2))
    in_maps = []
    for c in range(N_CORES):
        b = c >> 2
        r0 = (c & 3) * ROWS_PER_CORE
        a5 = (d0[b].reshape(D, IJ)[:, r0:r0 + ROWS_PER_CORE] * np.float32(5.0))
        a5q = np.ascontiguousarray(a5.astype(ml_dtypes.float8_e4m3fn))
        bmq = np.ascontiguousarray(
            d1[b].reshape(D, IJ).astype(ml_dtypes.float8_e4m3fn)
        )
        mk = mkv[b].reshape(IJ, IJ)[r0:r0 + ROWS_PER_CORE]  # [1024, 4096] bool
        # [row(G,128), col(2,2048)] -> [128, hg=(G,2), 2048]
        m4k = (mk.astype(np.float32) * np.float32(C)).reshape(G, 128, 2, HG_COLS)
        m8 = np.ascontiguousarray(m4k.transpose(1, 0, 2, 3).reshape(
            128, N_HG, HG_COLS
        )).astype(ml_dtypes.float8_e5m2)
        in_maps.append({"a5": a5q, "bm": bmq, "idn": idn, "m8": m8})
    return in_maps


def _run(in_maps, **kwargs):
    if "nc" not in _cached:
        _cached["nc"] = _build_program()
    return run_bass_kernel_spmd(_cached["nc"], in_maps, list(range(N_CORES)), **kwargs)


def _combine(results):
    total = 0.0
    n_hg = 128 * HG_COLS
    for r in results:
        accs = r["accs"].astype(np.float64)      # [128, 2*N_HG]
        A = accs[:, :N_HG].sum()
        Bv = -accs[:, N_HG:].sum() - THR * n_hg * N_HG
        total += (A + 250.0 * Bv) / 5.0
    return np.float32(total / float(B * IJ * IJ))


def kernel(descriptors_0, descriptors_1, similarity_mask):
    in_maps = _prep_inputs(descriptors_0, descriptors_1, similarity_mask)
    res = _run(in_maps)
    return _combine(res.results)


# revision 15
# speedup vs baseline: 1.1241x; 1.1241x over previous
"""DescriptorLoss kernel for Trainium2 (8 NeuronCores, SPMD data-parallel).

Math:
    d[b,ij,kl] = sum_c desc0[b,c,ij] * desc1[b,c,kl]
    loss = mean(where(mask, 250*relu(1 - d), relu(d - 0.2)))

Per core: shard = (batch, i-slab) -> 1024 ij rows x 4096 kl cols, processed
as 16 half-groups (hg) of 128 rows x 2048 cols (one 4-bank PSUM tile each).
The PE computes d' = 5*d with fp8e4m3 matmuls (same PE rate as bf16, half
the DMA/SBUF; quantization error ~2e-4 << tolerance).

Every accumulating op on TRN2 runs at 1 elem/lane/cycle (reductions have no
DVE perf-mode uops), so the two hinge sums per element are streamed straight
from PSUM. Measured pitfalls encoded here: two engines reading the SAME
PSUM banks contend (+20% each) or get serialized by Tile, so half-groups
are typed - each PSUM tile has exactly ONE consumer engine and the engines
overlap across consecutive half-groups:

ACT-hgs (even): PE injects u = d' - 2048*m (diag(-1) @ m8, fp8e5m2); with
  |d'| << 2048 the hinges separate on scalar thresholds:
      accA = sum relu(u - 1)       -> A directly     (masked die)
      accB = sum relu(-u - 2043)   -> B directly     (unmasked die)
DVE-hgs (odd): no inject; scalar_tensor_tensor with mask-encoded bounds:
      acc1 = sum min(max(d',1), X)  X = 1  if m else  C  -> A = acc1 - N
      acc2 = sum max(min(d',5), Y)  Y = -C if m else  5  -> B = 5N - acc2
Host: loss = sum(A + 250*B)/5 / count. Accumulators are fp32.
"""

import numpy as np
import ml_dtypes

import concourse.bacc as bacc
import concourse.mybir as mybir
import concourse.tile as tile
from concourse.bass_utils import run_bass_kernel_spmd

B, D, H, W = 2, 128, 64, 64
N_CORES = 8
IJ = H * W                 # 4096
ROWS_PER_CORE = IJ // 4    # 1024
G = ROWS_PER_CORE // 128   # 8 row groups of 128
HG_COLS = 2048             # half-group column width (one 4-bank PSUM tile)
N_HG = G * 2               # 16 half-groups per core
C = 2048.0                 # mask inject / clamp magnitude
THR = C - 5.0              # 2043: ACT B-hinge bias

MMF = 512                  # matmul moving free dim (cols per matmul)

_cached = {}


def _build_program():
    nc = bacc.Bacc("TRN2")
    f32 = mybir.dt.float32
    bf16 = mybir.dt.bfloat16
    f8e4 = mybir.dt.float8e4
    f8e5 = mybir.dt.float8e5
    Alu = mybir.AluOpType
    Act = mybir.ActivationFunctionType

    a5 = nc.declare_dram_parameter("a5", [D, G, 128], f8e4, isOutput=False)
    bm = nc.declare_dram_parameter("bm", [D, IJ], f8e4, isOutput=False)
    idn = nc.declare_dram_parameter("idn", [D, D], f8e5, isOutput=False)
    m8 = nc.declare_dram_parameter(
        "m8", [128, N_HG // 2, HG_COLS], f8e5, isOutput=False
    )
    x8 = nc.declare_dram_parameter(
        "x8", [128, N_HG // 2, HG_COLS], f8e5, isOutput=False
    )
    y8 = nc.declare_dram_parameter(
        "y8", [128, N_HG // 2, HG_COLS], f8e5, isOutput=False
    )
    accs_out = nc.declare_dram_parameter("accs", [128, 2 * N_HG], f32, isOutput=True)

    with tile.TileContext(nc) as tc:
        with (
            tc.tile_pool(name="desc", bufs=1) as desc_pool,
            tc.tile_pool(name="mask", bufs=6) as mask_pool,
            tc.tile_pool(name="gouta", bufs=2) as ga_pool,
            tc.tile_pool(name="goutb", bufs=2) as gb_pool,
            tc.tile_pool(name="accs", bufs=1) as acc_pool,
            tc.tile_pool(name="psd", bufs=2, space="PSUM") as psum_pool,
        ):
            a5_t = desc_pool.tile([D, G, 128], f8e4, tag="a5")
            bm_t = desc_pool.tile([D, IJ], f8e4, tag="bm")
            id_t = desc_pool.tile([D, D], f8e5, tag="idn")
            bias_a = desc_pool.tile([128, 1], f32, tag="ba")
            bias_b = desc_pool.tile([128, 1], f32, tag="bb")
            # split the big input DMAs so the first matmuls can start early
            nc.sync.dma_start(a5_t[:, 0, :], a5[:, 0, :])
            nc.sync.dma_start(bm_t[:, :HG_COLS], bm[:, :HG_COLS])
            nc.sync.dma_start(id_t[:], idn[:])
            nc.sync.dma_start(bm_t[:, HG_COLS:], bm[:, HG_COLS:])
            nc.sync.dma_start(a5_t[:, 1:, :], a5[:, 1:, :])
            nc.gpsimd.memset(bias_a[:], -1.0)
            nc.gpsimd.memset(bias_b[:], -THR)

            accA_t = acc_pool.tile([128, N_HG], f32, tag="accsA")
            accB_t = acc_pool.tile([128, N_HG], f32, tag="accsB")

            for hg in range(N_HG):
                g, h = hg // 2, hg % 2
                c0 = h * HG_COLS
                on_dve = hg % 2 == 1
                i2 = hg // 2

                if on_dve:
                    xm_t = mask_pool.tile([128, HG_COLS], f8e5, tag="x8")
                    ym_t = mask_pool.tile([128, HG_COLS], f8e5, tag="y8")
                    nc.sync.dma_start(xm_t[:], x8[:, i2, :])
                    nc.sync.dma_start(ym_t[:], y8[:, i2, :])
                else:
                    mm_t = mask_pool.tile([128, HG_COLS], f8e5, tag="m8")
                    nc.sync.dma_start(mm_t[:], m8[:, i2, :])

                psum_t = psum_pool.tile([128, HG_COLS], f32, tag="d")
                for j in range(HG_COLS // MMF):
                    js = slice(j * MMF, (j + 1) * MMF)
                    cs = slice(c0 + j * MMF, c0 + (j + 1) * MMF)
                    nc.tensor.matmul(
                        psum_t[:, js], a5_t[:, g, :], bm_t[:, cs],
                        start=True, stop=on_dve,
                    )
                if not on_dve:
                    for j in range(HG_COLS // MMF):
                        js = slice(j * MMF, (j + 1) * MMF)
                        nc.tensor.matmul(
                            psum_t[:, js], id_t[:], mm_t[:, js],
                            start=False, stop=True,
                        )

                if on_dve:
                    g1 = gb_pool.tile([128, HG_COLS], bf16, tag="gb")
                    g2 = gb_pool.tile([128, HG_COLS], bf16, tag="gb")
                    nc.vector.scalar_tensor_tensor(
                        g1[:], psum_t[:], 1.0, xm_t[:],
                        op0=Alu.max, op1=Alu.min,
                        accum_out=accA_t[:, hg:hg + 1],
                    )
                    nc.vector.scalar_tensor_tensor(
                        g2[:], psum_t[:], 5.0, ym_t[:],
                        op0=Alu.min, op1=Alu.max,
                        accum_out=accB_t[:, hg:hg + 1],
                    )
                else:
                    g1 = ga_pool.tile([128, HG_COLS], bf16, tag="ga")
                    g2 = ga_pool.tile([128, HG_COLS], bf16, tag="ga")
                    nc.scalar.activation(
                        g1[:], psum_t[:], Act.Relu,
                        bias=bias_a[:], scale=1.0,
                        accum_out=accA_t[:, hg:hg + 1],
                    )
                    nc.scalar.activation(
                        g2[:], psum_t[:], Act.Relu,
                        bias=bias_b[:], scale=-1.0,
                        accum_out=accB_t[:, hg:hg + 1],
                    )

            nc.sync.dma_start(accs_out[:, :N_HG], accA_t[:])
            nc.sync.dma_start(accs_out[:, N_HG:], accB_t[:])

    nc.finalize()
    return nc


def _prep_inputs(descriptors_0, descriptors_1, similarity_mask):
    d0 = np.asarray(descriptors_0, dtype=np.float32)
    d1 = np.asarray(descriptors_1, dtype=np.float32)
    mkv = np.asarray(similarity_mask)
    Cf = np.float32(C)
    idn = np.zeros((D, D), dtype=np.float32)
    np.fill_diagonal(idn, -1.0)
    idn = np.ascontiguousarray(idn.astype(ml_dtypes.float8_e5m2))
    in_maps = []
    for c in range(N_CORES):
        b = c >> 2
        r0 = (c & 3) * ROWS_PER_CORE
        a5 = (d0[b].reshape(D, IJ)[:, r0:r0 + ROWS_PER_CORE] * np.float32(5.0))
        a5q = np.ascontiguousarray(
            a5.reshape(D, G, 128).astype(ml_dtypes.float8_e4m3fn)
        )
        bmq = np.ascontiguousarray(
            d1[b].reshape(D, IJ).astype(ml_dtypes.float8_e4m3fn)
        )
        mk = mkv[b].reshape(IJ, IJ)[r0:r0 + ROWS_PER_CORE]  # [1024, 4096] bool
        m8l, x8l, y8l = [], [], []
        for hg in range(N_HG):
            g, h = hg // 2, hg % 2
            blk = mk[g * 128:(g + 1) * 128, h * HG_COLS:(h + 1) * HG_COLS]
            if hg % 2 == 1:
                x8l.append(np.where(blk, np.float32(1.0), Cf))
                y8l.append(np.where(blk, -Cf, np.float32(5.0)))
            else:
                m8l.append(blk.astype(np.float32) * Cf)
        in_maps.append(
            {
                "a5": a5q,
                "bm": bmq,
                "idn": idn,
                "m8": np.ascontiguousarray(np.stack(m8l, axis=1)).astype(
                    ml_dtypes.float8_e5m2
                ),
                "x8": np.ascontiguousarray(np.stack(x8l, axis=1)).astype(
                    ml_dtypes.float8_e5m2
                ),
                "y8": np.ascontiguousarray(np.stack(y8l, axis=1)).astype(
                    ml_dtypes.float8_e5m2
                ),
            }
        )
    return in_maps


def _run(in_maps, **kwargs):
    if "nc" not in _cached:
        _cached["nc"] = _build_program()
    return run_bass_kernel_spmd(_cached["nc"], in_maps, list(range(N_CORES)), **kwargs)


def _combine(results):
    total = 0.0
    n_hg = 128 * HG_COLS
    for r in results:
        accs = r["accs"].astype(np.float64)      # [128, 2*N_HG]
        accA = accs[:, :N_HG]
        accB = accs[:, N_HG:]
        for hg in range(N_HG):
            a1 = accA[:, hg].sum()
            a2 = accB[:, hg].sum()
            if hg % 2 == 1:
                A = a1 - n_hg
                Bv = 5.0 * n_hg - a2
            else:
                A = a1
                Bv = a2
            total += (A + 250.0 * Bv) / 5.0
    return np.float32(total / float(B * IJ * IJ))


def kernel(descriptors_0, descriptors_1, similarity_mask):
    in_maps = _prep_inputs(descriptors_0, descriptors_1, similarity_mask)
    res = _run(in_maps)
    return _combine(res.results)


# revision 18
# speedup vs baseline: 1.5261x; 1.3577x over previous
"""DescriptorLoss kernel for Trainium2 (8 NeuronCores, SPMD data-parallel).

Math:
    d[b,ij,kl] = sum_c desc0[b,c,ij] * desc1[b,c,kl]
    loss = mean(where(mask, 250*relu(1 - d), relu(d - 0.2)))

Per core: shard = (batch, i-slab) -> 1024 ij rows x 4096 kl cols, processed
as 16 half-groups (hg) of 128 rows x 2048 cols (one 4-bank PSUM tile each).
The PE computes d' = 5*d with fp8e4m3 matmuls (same PE rate as bf16, less
DMA/SBUF; quantization error ~2e-4 << tolerance) and injects the mask with
one extra matmul per 512 cols:  u = d' - 2048*m  (diag(-1) @ m8, fp8e5m2),
putting both hinges in disjoint scalar ranges (|d'| < ~1100 << 2048).

Reductions on TRN2 run at 1 elem/lane/cycle on ACT/DVE only (no DVE
perf-mode uops for accumulating ops; GPSIMD cannot touch PSUM), and two
engines reading the same PSUM banks contend (+20%) or serialize. So each
PSUM tile is read EXACTLY ONCE, by ACT, with an information-preserving
Leaky-ReLU that simultaneously evaluates the positive hinge:

  ACT pass1 (PSUM): out1 = |u - 1| fp16->SBUF, acc1 = sum|u-1|
      A = sum relu(u-1) = (Su - N)/2 + acc1/2   via relu(x) = (x+|x|)/2;
      Su = sum(u) computed EXACTLY on the host from the quantized inputs
      (a5q . colsum(bmq) - 2048*popcount, a few MFLOP).
  DVE pass2 (SBUF, decoupled from PSUM): acc2 = sum max(out1, 2044)
      B = sum relu(-u-2043) = acc2 - 2044*N   (masked: |u-1| = 1-u so
      |u-1|-2044 = -u-2043; unmasked: |u-1| < 1200 < 2044 clamps away).

Host: loss = sum(A + 250*B)/5 / count.  PSUM hold is a single 1.97us pass,
so the 2-buffer PSUM rotation sustains ACT at ~full rate while DVE trails
one stage behind on SBUF data; PE (128 small matmuls) fills the gaps and
stays un-throttled.
"""

import numpy as np
import ml_dtypes

import concourse.bacc as bacc
import concourse.mybir as mybir
import concourse.tile as tile
from concourse.bass_utils import run_bass_kernel_spmd

B, D, H, W = 2, 128, 64, 64
N_CORES = 8
IJ = H * W                 # 4096
ROWS_PER_CORE = IJ // 4    # 1024
G = ROWS_PER_CORE // 128   # 8 row groups of 128
HG_COLS = 2048             # half-group column width (one 4-bank PSUM tile)
N_HG = G * 2               # 16 half-groups per core
C = 2048.0                 # mask inject magnitude
THR2 = C - 4.0             # 2044: pass2 clamp constant (fp16-exact)

MMF = 512                  # matmul moving free dim (cols per matmul)

_cached = {}


def _build_program():
    nc = bacc.Bacc("TRN2")
    f32 = mybir.dt.float32
    f16 = mybir.dt.float16
    f8e4 = mybir.dt.float8e4
    f8e5 = mybir.dt.float8e5
    Alu = mybir.AluOpType
    Act = mybir.ActivationFunctionType

    a5 = nc.declare_dram_parameter("a5", [D, G, 128], f8e4, isOutput=False)
    bm = nc.declare_dram_parameter("bm", [D, IJ], f8e4, isOutput=False)
    idn = nc.declare_dram_parameter("idn", [D, D], f8e5, isOutput=False)
    m8 = nc.declare_dram_parameter("m8", [128, N_HG, HG_COLS], f8e5, isOutput=False)
    accs_out = nc.declare_dram_parameter("accs", [128, 2 * N_HG], f32, isOutput=True)

    with tile.TileContext(nc) as tc:
        with (
            tc.tile_pool(name="desc", bufs=1) as desc_pool,
            tc.tile_pool(name="mask", bufs=4) as mask_pool,
            tc.tile_pool(name="out1", bufs=4) as o1_pool,
            tc.tile_pool(name="gout", bufs=2) as g_pool,
            tc.tile_pool(name="accs", bufs=1) as acc_pool,
            tc.tile_pool(name="psd", bufs=2, space="PSUM") as psum_pool,
        ):
            a5_t = desc_pool.tile([D, G, 128], f8e4, tag="a5")
            bm_t = desc_pool.tile([D, IJ], f8e4, tag="bm")
            id_t = desc_pool.tile([D, D], f8e5, tag="idn")
            bias_a = desc_pool.tile([128, 1], f32, tag="ba")
            # split the big input DMAs so the first matmuls can start early
            nc.sync.dma_start(a5_t[:, 0, :], a5[:, 0, :])
            nc.sync.dma_start(bm_t[:, :HG_COLS], bm[:, :HG_COLS])
            nc.sync.dma_start(id_t[:], idn[:])
            nc.sync.dma_start(bm_t[:, HG_COLS:], bm[:, HG_COLS:])
            nc.sync.dma_start(a5_t[:, 1:, :], a5[:, 1:, :])
            nc.gpsimd.memset(bias_a[:], -1.0)

            accA_t = acc_pool.tile([128, N_HG], f32, tag="accsA")
            accB_t = acc_pool.tile([128, N_HG], f32, tag="accsB")

            for hg in range(N_HG):
                g, h = hg // 2, hg % 2
                c0 = h * HG_COLS

                mm_t = mask_pool.tile([128, HG_COLS], f8e5, tag="m8")
                nc.sync.dma_start(mm_t[:], m8[:, hg, :])

                psum_t = psum_pool.tile([128, HG_COLS], f32, tag="d")
                for j in range(HG_COLS // MMF):
                    js = slice(j * MMF, (j + 1) * MMF)
                    cs = slice(c0 + j * MMF, c0 + (j + 1) * MMF)
                    nc.tensor.matmul(
                        psum_t[:, js], a5_t[:, g, :], bm_t[:, cs],
                        start=True, stop=False,
                    )
                for j in range(HG_COLS // MMF):
                    js = slice(j * MMF, (j + 1) * MMF)
                    nc.tensor.matmul(
                        psum_t[:, js], id_t[:], mm_t[:, js],
                        start=False, stop=True,
                    )

                out1 = o1_pool.tile([128, HG_COLS], f16, tag="o1")
                nc.scalar.activation(
                    out1[:], psum_t[:], Act.Abs,
                    bias=bias_a[:], scale=1.0,
                    accum_out=accA_t[:, hg:hg + 1],
                )
                g2 = g_pool.tile([128, HG_COLS], f16, tag="g")
                nc.vector.tensor_scalar(
                    g2[:], out1[:], THR2, None,
                    op0=Alu.max, op1=Alu.add,
                    accum_out=accB_t[:, hg:hg + 1],
                )

            nc.sync.dma_start(accs_out[:, :N_HG], accA_t[:])
            nc.sync.dma_start(accs_out[:, N_HG:], accB_t[:])

    nc.finalize()
    return nc


def _prep_inputs(descriptors_0, descriptors_1, similarity_mask):
    d0 = np.asarray(descriptors_0, dtype=np.float32)
    d1 = np.asarray(descriptors_1, dtype=np.float32)
    mkv = np.asarray(similarity_mask)
    idn = np.zeros((D, D), dtype=np.float32)
    np.fill_diagonal(idn, -1.0)
    idn = np.ascontiguousarray(idn.astype(ml_dtypes.float8_e5m2))
    in_maps = []
    su_list = []
    for c in range(N_CORES):
        b = c >> 2
        r0 = (c & 3) * ROWS_PER_CORE
        a5 = (d0[b].reshape(D, IJ)[:, r0:r0 + ROWS_PER_CORE] * np.float32(5.0))
        a5q8 = a5.astype(ml_dtypes.float8_e4m3fn)
        a5q = a5q8.astype(np.float32)            # [128 chan, 1024 rows]
        bmq8 = d1[b].reshape(D, IJ).astype(ml_dtypes.float8_e4m3fn)
        bmq = bmq8.astype(np.float32)
        mk = mkv[b].reshape(IJ, IJ)[r0:r0 + ROWS_PER_CORE]  # [1024, 4096] bool
        # [row(G,128), col(2,2048)] -> [128, hg=(G,2), 2048]
        m4k = (mk.astype(np.float32) * np.float32(C)).reshape(G, 128, 2, HG_COLS)
        m8v = np.ascontiguousarray(m4k.transpose(1, 0, 2, 3).reshape(
            128, N_HG, HG_COLS
        )).astype(ml_dtypes.float8_e5m2)
        # host-exact Su_total = sum(u) over the whole slab
        bsum = bmq.sum(axis=1, dtype=np.float64)
        su_tot = float((a5q.astype(np.float64).T @ bsum).sum()) - float(C) * float(
            mk.sum(dtype=np.int64)
        )
        su_list.append(su_tot)
        in_maps.append(
            {
                "a5": np.ascontiguousarray(a5q8.reshape(D, G, 128)),
                "bm": np.ascontiguousarray(bmq8),
                "idn": idn,
                "m8": m8v,
            }
        )
    _cached["su"] = su_list
    return in_maps


def _run(in_maps, **kwargs):
    if "nc" not in _cached:
        _cached["nc"] = _build_program()
    return run_bass_kernel_spmd(_cached["nc"], in_maps, list(range(N_CORES)), **kwargs)


def _combine(results):
    su_list = _cached["su"]
    n_core = ROWS_PER_CORE * IJ                  # elements per core
    total = 0.0
    for c, r in enumerate(results):
        accs = r["accs"].astype(np.float64)      # [128, 2*N_HG]
        acc1 = accs[:, :N_HG].sum()
        acc2 = accs[:, N_HG:].sum()
        A = 0.5 * (su_list[c] - n_core) + 0.5 * acc1
        Bv = acc2 - THR2 * n_core
        total += (A + 250.0 * Bv) / 5.0
    return np.float32(total / float(B * IJ * IJ))


def kernel(descriptors_0, descriptors_1, similarity_mask):
    in_maps = _prep_inputs(descriptors_0, descriptors_1, similarity_mask)
    res = _run(in_maps)
    return _combine(res.results)
